# revision 4
# baseline (speedup 1.0000x reference)
"""HGT (heterogeneous graph transformer) Bass kernel for 8 Trainium2 NeuronCores.

v2 redesign vs the per-tile-indirect-DMA baseline:
  - Destination-ownership sharding (unchanged): core c owns 1/8 of each node
    type; every edge lives on the core owning its destination, so
    segment-softmax and aggregation are core-local.
  - Batched SWDGE gathers: per-edge source features and destination Q rows are
    fetched with a handful of dma_gather instructions (thousands of rows each)
    instead of one 128-row indirect DMA per tile. Source tables for the
    user-typed relations are host-compacted per core (unique sources only) so
    indices fit dma_gather's int16 format.
  - bf16 tables and matmuls everywhere (fp32 PSUM accumulation); transposing
    gathers deliver x^T / q^T tiles directly, eliminating per-tile PE
    transposes.
  - Scores on the PE: kT = Wk_fold^T-free matmul, per-edge dot via a head-mask
    matmul; exp on ACT straight into the message tile; alpha*v via one fused
    scalar_tensor_tensor; aggregation via one-hot matmul into PSUM.
  - Q projection (phase 1) and the output transform (gelu/Wa/skip/relu) are
    fused around the edge phase; node chunks move with one DMA per chunk.
  - Two executions of ONE compiled single-layer program; the host performs the
    inter-layer exchange by concatenating returned owned slices.
"""
import sys

sys.path.insert(0, "/opt/trn_rl_repo")

import numpy as np
import ml_dtypes

import concourse.bass as bass
import concourse.mybir as mybir
import concourse.tile as tile
from concourse import bacc
from concourse import library_config
from concourse.bass_utils import run_bass_kernel_spmd
from concourse.masks import make_identity

# ---------------- problem constants ----------------
N_USER, N_NEWS = 100000, 20000
C, H, NL = 128, 4, 2
D = C // H
EDGE_SRC_DST = ((0, 1), (1, 0), (0, 0))  # relation -> (src_type, dst_type)
SIZES = (N_USER, N_NEWS)
M = 8
OWN = (N_USER // M, N_NEWS // M)               # (12500, 2500)
NBINS = tuple((o + 127) // 128 for o in OWN)   # (98, 20)
PADN = tuple(nb * 128 for nb in NBINS)         # (12544, 2560)
F32, BF16, I16 = mybir.dt.float32, mybir.dt.bfloat16, mybir.dt.int16
EPS = 1e-16
BF = ml_dtypes.bfloat16

CH_R0 = 2     # news bins per gather chunk (nt ~13 tiles/bin)
CH_R12 = 8    # user bins per gather chunk
CH_P1 = 8     # bins per phase-1 / epilogue transfer chunk
OH_ENG = "vector"   # engine for the one-hot build: "gpsimd" or "vector"
ACT_GELU = mybir.ActivationFunctionType.Gelu   # debug_sim swaps to Tanh
ABLATE = 0          # 0=full, 1=gathers only (no tile compute), 2=phase1 only

# ---------------- host-side prep ----------------


def fold_weights(inp):
    Wk, bk = np.asarray(inp["Wk"]), np.asarray(inp["bk"])
    Wq, bq = np.asarray(inp["Wq"]), np.asarray(inp["bq"])
    Wv, bv = np.asarray(inp["Wv"]), np.asarray(inp["bv"])
    Wa, ba = np.asarray(inp["Wa"]), np.asarray(inp["ba"])
    skip = np.asarray(inp["skip"])
    a_rel, m_rel, p_rel = (np.asarray(inp[k]) for k in ("a_rel", "m_rel", "p_rel"))
    assert abs(bk).max() == 0 and abs(bq).max() == 0, "nonzero biases unsupported"
    assert abs(bv).max() == 0 and abs(ba).max() == 0, "nonzero biases unsupported"
    inv_sqrt_d = 1.0 / np.sqrt(D)
    W = {}
    for l in range(NL):
        for r, (st, dt) in enumerate(EDGE_SRC_DST):
            scale = p_rel[l, r] * inv_sqrt_d
            bd_a = np.zeros((C, C), np.float32)
            bd_m = np.zeros((C, C), np.float32)
            for h in range(H):
                s = slice(h * D, (h + 1) * D)
                bd_a[s, s] = a_rel[l, r, h] * scale[h]
                bd_m[s, s] = m_rel[l, r, h]
            W[("wk", l, r)] = (Wk[l, st] @ bd_a).astype(BF)
            W[("wv", l, r)] = (Wv[l, st] @ bd_m).astype(BF)
        for t in range(2):
            a = 1.0 / (1.0 + np.exp(-float(skip[l, t])))
            W[("wq", l, t)] = Wq[l, t].astype(BF)
            W[("wa", l, t)] = (Wa[l, t] * a).astype(BF)
            W[("oma", l, t)] = np.float32(1.0 - a)
    return W


def _balanced_bins(degs, nbins):
    """old_local -> bin*128 + slot, snake assignment of degree-sorted nodes."""
    n = len(degs)
    order = np.argsort(-degs, kind="stable")
    perm = np.empty(n, np.int64)
    slot_count = np.zeros(nbins, np.int64)
    fwd = list(range(nbins))
    rev = fwd[::-1]
    seq = []
    while len(seq) < n:
        seq.extend(fwd)
        if len(seq) < n:
            seq.extend(rev)
    for i in range(n):
        b = seq[i]
        perm[order[i]] = b * 128 + slot_count[b]
        slot_count[b] += 1
    assert slot_count.max() <= 128
    return perm


def _wrap16(idx):
    """[S] int -> [128, S//16] int16 wrapped layout for dma_gather
    (16-partition wrap, replicated across the 8 Q7 cores)."""
    idx = np.asarray(idx)
    S = len(idx)
    assert S % 16 == 0
    cols = S // 16
    w = idx.astype(np.int16).reshape(cols, 16).T
    return np.ascontiguousarray(np.tile(w, (8, 1)))


def build_schedule(inp):
    eis = [np.asarray(inp[k]) for k in ("ei_posts", "ei_rev", "ei_follows")]
    # per (core, type) in-degree (summed over relations) for balancing
    deg = [[np.zeros(OWN[t], np.int64) for t in range(2)] for _ in range(M)]
    for r, (st, dt) in enumerate(EDGE_SRC_DST):
        dst = eis[r][1].astype(np.int64)
        core_of = dst // OWN[dt]
        loc = dst - core_of * OWN[dt]
        for c in range(M):
            deg[c][dt] += np.bincount(loc[core_of == c], minlength=OWN[dt])
    perms = [[_balanced_bins(deg[c][t], NBINS[t]) for t in range(2)] for c in range(M)]

    def perm_global(t):
        g = np.empty(SIZES[t], np.int64)
        for c in range(M):
            o = OWN[t]
            g[c * o:(c + 1) * o] = c * PADN[t] + perms[c][t]
        return g

    pg = [perm_global(0), perm_global(1)]

    # per (core, relation): edges sorted by destination bin
    edges = [[None] * 3 for _ in range(M)]   # (src_raw, dloc, counts_per_bin)
    for r, (st, dt) in enumerate(EDGE_SRC_DST):
        src = eis[r][0].astype(np.int64)
        dst = eis[r][1].astype(np.int64)
        core_of = dst // OWN[dt]
        for c in range(M):
            m = core_of == c
            s_c = src[m]
            dloc = perms[c][dt][dst[m] - c * OWN[dt]]
            b_c = dloc // 128
            order = np.argsort(b_c, kind="stable")
            s_c, dloc, b_c = s_c[order], dloc[order], b_c[order]
            counts = np.bincount(b_c, minlength=NBINS[dt])
            edges[c][r] = (s_c, dloc, counts)

    # per-bin tile counts: max over cores
    nt_b = []
    for r, (st, dt) in enumerate(EDGE_SRC_DST):
        cnts = np.stack([edges[c][r][2] for c in range(M)])  # [M, nbins]
        nt = np.maximum(1, -(-cnts.max(axis=0) // 128)).astype(np.int64)
        nt_b.append(nt)
    tile_off = [np.concatenate([[0], np.cumsum(nt_b[r])]) for r in range(3)]

    # compacted source tables for user-src relations (r0, r2)
    uniq = [[None] * 3 for _ in range(M)]
    NT = [0, 0, 0]
    for r in (0, 2):
        for c in range(M):
            u = np.unique(edges[c][r][0])
            uniq[c][r] = u
            NT[r] = max(NT[r], len(u))
        NT[r] = -(-NT[r] // 16) * 16
        assert NT[r] < 32767, f"compact table too large for int16: {NT[r]}"

    # per (core, relation) slot arrays
    cores = []
    for c in range(M):
        per_rel = []
        for r, (st, dt) in enumerate(EDGE_SRC_DST):
            s_c, dloc, counts = edges[c][r]
            if r == 1:
                sidx_all = pg[1][s_c]           # global permuted news id
            else:
                sidx_all = np.searchsorted(uniq[c][r], s_c)
            T = int(tile_off[r][-1])
            S = T * 128
            sidx = np.zeros(S, np.int64)
            qidx = np.zeros(S, np.int64)
            dcv = np.full(S, -1.0, np.float32)
            pos = 0
            for b in range(NBINS[dt]):
                n_e = int(counts[b])
                o = int(tile_off[r][b]) * 128
                sidx[o:o + n_e] = sidx_all[pos:pos + n_e]
                qidx[o:o + n_e] = dloc[pos:pos + n_e]
                dcv[o:o + n_e] = (dloc[pos:pos + n_e] % 128).astype(np.float32)
                pos += n_e
            assert sidx.max() < 32767 and qidx.max() < 32767
            per_rel.append(dict(
                sidx=_wrap16(sidx), qidx=_wrap16(qidx),
                dc=np.ascontiguousarray(dcv.reshape(T, 128).T),   # [128, T]
            ))
        cores.append(per_rel)

    return dict(perms=perms, pg=pg, nt_b=nt_b, tile_off=tile_off,
                uniq=uniq, NT=NT, cores=cores)


# ---------------- device program ----------------


def build_program(sched):
    nt_b = sched["nt_b"]
    tile_off = sched["tile_off"]
    NT0, NT2 = sched["NT"][0], sched["NT"][2]
    T_r = [int(tile_off[r][-1]) for r in range(3)]

    nc = bacc.Bacc("TRN2", target_bir_lowering=False, debug=False)

    tb0 = nc.dram_tensor("tb0", [NT0, C], BF16, kind="ExternalInput")
    tbn = nc.dram_tensor("tbn", [M * PADN[1], C], BF16, kind="ExternalInput")
    tb2 = nc.dram_tensor("tb2", [NT2, C], BF16, kind="ExternalInput")
    xo = [nc.dram_tensor(f"xo{t}", [PADN[t], C], F32, kind="ExternalInput")
          for t in range(2)]
    qtab = [nc.dram_tensor(f"qtab{t}", [PADN[t], C], BF16) for t in range(2)]
    wk = [nc.dram_tensor(f"wk{r}", [C, C], BF16, kind="ExternalInput")
          for r in range(3)]
    wv = [nc.dram_tensor(f"wv{r}", [C, C], BF16, kind="ExternalInput")
          for r in range(3)]
    wq = [nc.dram_tensor(f"wq{t}", [C, C], BF16, kind="ExternalInput")
          for t in range(2)]
    wa = [nc.dram_tensor(f"wa{t}", [C, C], BF16, kind="ExternalInput")
          for t in range(2)]
    hm = nc.dram_tensor("hm", [C, H], BF16, kind="ExternalInput")
    iota = nc.dram_tensor("iota", [128, 128], F32, kind="ExternalInput")
    oma = nc.dram_tensor("oma", [128, 2], F32, kind="ExternalInput")
    sidx = [nc.dram_tensor(f"sidx{r}", [128, T_r[r] * 8], I16,
                           kind="ExternalInput") for r in range(3)]
    qidx = [nc.dram_tensor(f"qidx{r}", [128, T_r[r] * 8], I16,
                           kind="ExternalInput") for r in range(3)]
    dcr = [nc.dram_tensor(f"dc{r}", [128, T_r[r]], F32, kind="ExternalInput")
           for r in range(3)]
    nx = [nc.dram_tensor(f"nx{t}", [PADN[t], C], F32, kind="ExternalOutput")
          for t in range(2)]
    xtab = (tb0, tbn, tb2)

    with tile.TileContext(nc) as tc:
        with tc.tile_pool(name="const", bufs=1) as constp:
            nc.gpsimd.load_library(library_config.mlp)
            ident = constp.tile([128, 128], F32)
            make_identity(nc, ident[:])
            iota_t = constp.tile([128, 128], F32)
            nc.sync.dma_start(out=iota_t[:], in_=iota[:])
            oma_t = constp.tile([128, 2], F32)
            nc.sync.dma_start(out=oma_t[:], in_=oma[:])
            hm_t = constp.tile([C, H], BF16)
            nc.sync.dma_start(out=hm_t[:], in_=hm[:])
            wk_t = [constp.tile([C, C], BF16, name=f"wk_t{r}", tag=f"wk{r}")
                    for r in range(3)]
            wv_t = [constp.tile([C, C], BF16, name=f"wv_t{r}", tag=f"wv{r}")
                    for r in range(3)]
            wq_t = [constp.tile([C, C], BF16, name=f"wq_t{t}", tag=f"wq{t}")
                    for t in range(2)]
            wa_t = [constp.tile([C, C], BF16, name=f"wa_t{t}", tag=f"wa{t}")
                    for t in range(2)]
            for r in range(3):
                nc.sync.dma_start(out=wk_t[r][:], in_=wk[r][:])
                nc.sync.dma_start(out=wv_t[r][:], in_=wv[r][:])
            for t in range(2):
                nc.sync.dma_start(out=wq_t[t][:], in_=wq[t][:])
                nc.sync.dma_start(out=wa_t[t][:], in_=wa[t][:])
            dc_t = [constp.tile([128, T_r[r]], F32, name=f"dc_t{r}", tag=f"dc{r}")
                    for r in range(3)]
            for r in range(3):
                nc.sync.dma_start(out=dc_t[r][:], in_=dcr[r][:])

            oh_eng = nc.gpsimd if OH_ENG == "gpsimd" else nc.vector

            # ---------- phase 1: Q tables ----------
            with tc.tile_pool(name="p1", bufs=2) as p1, \
                 tc.tile_pool(name="p1ps", bufs=2, space="PSUM") as p1ps:
                for t in range(2):
                    for b0 in range(0, NBINS[t], CH_P1):
                        nb = min(CH_P1, NBINS[t] - b0)
                        rows = slice(b0 * 128, (b0 + nb) * 128)
                        xoc = p1.tile([128, nb * C], F32, tag="xoc")
                        nc.sync.dma_start(
                            out=xoc[:].rearrange("p (c f) -> p c f", f=C),
                            in_=xo[t][rows].rearrange("(c p) f -> p c f", p=128))
                        qc = p1.tile([128, nb * C], BF16, tag="qc")
                        for cb in range(nb):
                            col = slice(cb * C, (cb + 1) * C)
                            xT_ps = p1ps.tile([128, 128], F32, tag="xT")
                            nc.tensor.transpose(out=xT_ps[:], in_=xoc[:, col],
                                                identity=ident[:])
                            xT16 = p1.tile([128, 128], BF16, tag="xT16")
                            nc.scalar.copy(out=xT16[:], in_=xT_ps[:])
                            q_ps = p1ps.tile([128, C], F32, tag="q")
                            nc.tensor.matmul(out=q_ps[:], lhsT=xT16[:],
                                             rhs=wq_t[t][:], start=True, stop=True)
                            nc.scalar.copy(out=qc[:, col], in_=q_ps[:])
                        nc.sync.dma_start(
                            out=qtab[t][rows].rearrange("(c p) f -> p c f", p=128),
                            in_=qc[:].rearrange("p (c f) -> p c f", f=C))

            tc.strict_bb_all_engine_barrier()

            # ---------- phase 2 ----------
            def tile_body(r, xg, qg, goff, w3p, php, acc, aoff, tcol, start, stop):
                col = slice(goff * 128, (goff + 1) * 128)
                kt_ps = php.tile([128, 128], F32, tag="kt")
                nc.tensor.matmul(out=kt_ps[:], lhsT=wk_t[r][:], rhs=xg[:, col],
                                 start=True, stop=True)
                prod = w3p.tile([128, 128], BF16, tag="prod")
                nc.vector.scalar_tensor_tensor(
                    out=prod[:], in0=kt_ps[:], scalar=1.0, in1=qg[:, col],
                    op0=mybir.AluOpType.mult, op1=mybir.AluOpType.mult)
                sc_ps = php.tile([128, H], F32, tag="sc", bufs=1)
                nc.tensor.matmul(out=sc_ps[:], lhsT=prod[:], rhs=hm_t[:],
                                 start=True, stop=True)
                w3 = w3p.tile([128, H * (D + 1)], BF16, tag="w3")
                w3h = w3[:].rearrange("p (h q) -> p h q", h=H)
                alpha = w3h[:, :, D:D + 1]
                nc.scalar.activation(out=alpha, in_=sc_ps[:],
                                     func=mybir.ActivationFunctionType.Exp)
                v_ps = php.tile([128, C], F32, tag="v", bufs=1)
                nc.tensor.matmul(out=v_ps[:], lhsT=xg[:, col], rhs=wv_t[r][:],
                                 start=True, stop=True)
                nc.vector.scalar_tensor_tensor(
                    out=w3h[:, :, 0:D],
                    in0=v_ps[:].rearrange("p (h d) -> p h d", h=H),
                    scalar=1.0, in1=alpha.broadcast_to((128, H, D)),
                    op0=mybir.AluOpType.mult, op1=mybir.AluOpType.mult)
                oh = w3p.tile([128, 128], BF16, tag="oh")
                oh_eng.tensor_scalar(
                    out=oh[:], in0=iota_t[:], scalar1=dc_t[r][:, tcol:tcol + 1],
                    scalar2=None, op0=mybir.AluOpType.is_equal)
                nc.tensor.matmul(out=acc[:, aoff:aoff + H * (D + 1)], lhsT=oh[:],
                                 rhs=w3[:], start=start, stop=stop)

            def epilogue(t, R, accs, ep, epps, xoc, outc, col):
                den = ep.tile([128, R * H], F32, tag="den")
                denv = den[:].rearrange("p (r h) -> p r h", r=R)
                for ri in range(R):
                    accv = accs[ri][:].rearrange("p (h q) -> p h q", h=H)
                    nc.vector.tensor_scalar(
                        out=denv[:, ri], in0=accv[:, :, D:D + 1], scalar1=EPS,
                        scalar2=None, op0=mybir.AluOpType.add)
                rec = ep.tile([128, R * H], F32, tag="rec")
                nc.vector.reciprocal(out=rec[:], in_=den[:])
                recv = rec[:].rearrange("p (r h) -> p r h", r=R)
                aggs = ep.tile([128, R * C], F32, tag="aggs")
                for ri in range(R):
                    accv = accs[ri][:].rearrange("p (h q) -> p h q", h=H)
                    nc.vector.scalar_tensor_tensor(
                        out=aggs[:, ri * C:(ri + 1) * C]
                            .rearrange("p (h d) -> p h d", h=H),
                        in0=accv[:, :, 0:D], scalar=1.0,
                        in1=recv[:, ri].unsqueeze(2).broadcast_to((128, H, D)),
                        op0=mybir.AluOpType.mult, op1=mybir.AluOpType.mult)
                if R == 2:
                    asum = ep.tile([128, C], F32, tag="asum")
                    nc.vector.tensor_tensor(out=asum[:], in0=aggs[:, 0:C],
                                            in1=aggs[:, C:2 * C],
                                            op=mybir.AluOpType.add)
                else:
                    asum = aggs
                gl = ep.tile([128, C], F32, tag="gl")
                nc.scalar.activation(out=gl[:], in_=asum[:, 0:C],
                                     func=ACT_GELU)
                glT_ps = epps.tile([128, 128], F32, tag="kt")
                nc.tensor.transpose(out=glT_ps[:], in_=gl[:], identity=ident[:])
                glT16 = ep.tile([128, 128], BF16, tag="glT16")
                nc.scalar.copy(out=glT16[:], in_=glT_ps[:])
                o_ps = epps.tile([128, C], F32, tag="v", bufs=1)
                nc.tensor.matmul(out=o_ps[:], lhsT=glT16[:], rhs=wa_t[t][:],
                                 start=True, stop=True)
                sk = ep.tile([128, C], F32, tag="sk")
                nc.vector.scalar_tensor_tensor(
                    out=sk[:], in0=xoc[:, col], scalar=oma_t[:, t:t + 1],
                    in1=o_ps[:], op0=mybir.AluOpType.mult,
                    op1=mybir.AluOpType.add)
                nc.scalar.activation(out=outc[:, col], in_=sk[:],
                                     func=mybir.ActivationFunctionType.Relu)

            def fetch_chunk(pool, iop, t, rels, b0, nb):
                """Issue gathers + node-chunk load for bins [b0, b0+nb)."""
                GSUB = 7    # tiles per dma_gather: 896 idx = 58 ring descs
                            # (SWDGE ring holds 64; self-triggered gathers
                            # deadlock when one instruction exceeds it)
                gt = {}
                for r in (rels if ABLATE < 2 else ()):
                    t0 = int(tile_off[r][b0])
                    t1 = int(tile_off[r][b0 + nb])
                    nt_c = t1 - t0
                    gn = nt_c * 128
                    si = pool.tile([128, nt_c * 8], I16, tag=f"si{r}")
                    nc.sync.dma_start(out=si[:], in_=sidx[r][:, t0 * 8:t1 * 8])
                    qi = pool.tile([128, nt_c * 8], I16, tag=f"qi{r}")
                    nc.sync.dma_start(out=qi[:], in_=qidx[r][:, t0 * 8:t1 * 8])
                    xg = pool.tile([128, gn], BF16, tag=f"xg{r}")
                    qg = pool.tile([128, gn], BF16, tag=f"qg{r}")
                    for s0 in range(0, nt_c, GSUB):
                        s1 = min(s0 + GSUB, nt_c)
                        n = (s1 - s0) * 128
                        nc.gpsimd.dma_gather(
                            xg[:, s0 * 128:s1 * 128].unsqueeze(1), xtab[r][:],
                            si[:, s0 * 8:s1 * 8], n, n, C, transpose=True)
                        nc.gpsimd.dma_gather(
                            qg[:, s0 * 128:s1 * 128].unsqueeze(1),
                            qtab[EDGE_SRC_DST[r][1]][:],
                            qi[:, s0 * 8:s1 * 8], n, n, C, transpose=True)
                    gt[r] = (xg, qg, t0)
                rows = slice(b0 * 128, (b0 + nb) * 128)
                xoc = iop.tile([128, nb * C], F32, tag="xoc")
                nc.sync.dma_start(
                    out=xoc[:].rearrange("p (c f) -> p c f", f=C),
                    in_=xo[t][rows].rearrange("(c p) f -> p c f", p=128))
                return gt, xoc

            def run_type(t, rels, chs, pool, iop, w3p, ep, php, ac):
                chunks = []
                nbins = NBINS[t]
                b0 = 0
                while b0 < nbins:
                    nb = min(chs, nbins - b0)
                    chunks.append((b0, nb))
                    b0 += nb
                R = len(rels)
                pref = fetch_chunk(pool, iop, t, rels, *chunks[0])
                for ci, (b0, nb) in enumerate(chunks):
                    gt, xoc = pref
                    if ci + 1 < len(chunks):
                        pref = fetch_chunk(pool, iop, t, rels, *chunks[ci + 1])
                    outc = iop.tile([128, nb * C], F32, tag="outc")
                    for bl in range(nb):
                        b = b0 + bl
                        if ABLATE >= 1:
                            nc.vector.tensor_copy(
                                out=outc[:, bl * C:(bl + 1) * C],
                                in_=xoc[:, bl * C:(bl + 1) * C])
                            continue
                        accs = [ac.tile([128, H * (D + 1)], F32,
                                        name=f"acc{ri}", tag=f"acc{ri}")
                                for ri in range(R)]
                        for ri, r in enumerate(rels):
                            xg, qg, t0 = gt[r]
                            nt = int(nt_b[r][b])
                            boff = int(tile_off[r][b])
                            for ti in range(nt):
                                tile_body(r, xg, qg, boff - t0 + ti, w3p, php,
                                          accs[ri], 0, boff + ti,
                                          ti == 0, ti == nt - 1)
                        epilogue(t, R, accs, ep, php, xoc, outc,
                                 slice(bl * C, (bl + 1) * C))
                    rows = slice(b0 * 128, (b0 + nb) * 128)
                    nc.sync.dma_start(
                        out=nx[t][rows].rearrange("(c p) f -> p c f", p=128),
                        in_=outc[:].rearrange("p (c f) -> p c f", f=C))

            # news (dst type 1): relation 0 only
            with tc.tile_pool(name="g0", bufs=2) as g0, \
                 tc.tile_pool(name="io0", bufs=2) as io0, \
                 tc.tile_pool(name="w30", bufs=3) as w30, \
                 tc.tile_pool(name="ep0", bufs=2) as ep0, \
                 tc.tile_pool(name="ph0", bufs=2, space="PSUM") as ph0, \
                 tc.tile_pool(name="ac0", bufs=2, space="PSUM") as ac0:
                run_type(1, (0,), CH_R0, g0, io0, w30, ep0, ph0, ac0)

            # user (dst type 0): relations 1 and 2
            with tc.tile_pool(name="g1", bufs=2) as g1, \
                 tc.tile_pool(name="io1", bufs=2) as io1, \
                 tc.tile_pool(name="w31", bufs=3) as w31, \
                 tc.tile_pool(name="ep1", bufs=2) as ep1, \
                 tc.tile_pool(name="ph1", bufs=2, space="PSUM") as ph1, \
                 tc.tile_pool(name="ac1", bufs=2, space="PSUM") as ac1:
                run_type(0, (1, 2), CH_R12, g1, io1, w31, ep1, ph1, ac1)

    nc.compile()
    return nc


# ---------------- kernel entry ----------------

TRACE = False
LAST_EXEC_NS = []
LAST_RES = None


def kernel(**inputs):
    inputs = {k: np.asarray(v) for k, v in inputs.items()}
    W = fold_weights(inputs)
    sched = build_schedule(inputs)
    pg = sched["pg"]
    nc = build_program(sched)
    core_ids = list(range(M))

    iota = np.tile(np.arange(128, dtype=np.float32)[None, :], (128, 1))
    hm = np.zeros((C, H), BF)
    for h in range(H):
        hm[h * D:(h + 1) * D, h] = 1.0

    # permuted global tables (f32), layer-1
    def permute_tables(x_user, x_news):
        tabs = []
        for t, x in ((0, x_user), (1, x_news)):
            tab = np.zeros((M * PADN[t], C), np.float32)
            tab[pg[t]] = x
            tabs.append(tab)
        return tabs

    x_user = np.asarray(inputs["x_user"], np.float32)
    x_news = np.asarray(inputs["x_news"], np.float32)
    xu_t, xn_t = permute_tables(x_user, x_news)

    for l in range(NL):
        tbn_g = xn_t.astype(BF)
        oma = np.stack([np.full(128, W[("oma", l, 0)], np.float32),
                        np.full(128, W[("oma", l, 1)], np.float32)], axis=1)
        in_maps = []
        for c in range(M):
            im = dict(
                tbn=tbn_g,
                xo0=np.ascontiguousarray(xu_t[c * PADN[0]:(c + 1) * PADN[0]]),
                xo1=np.ascontiguousarray(xn_t[c * PADN[1]:(c + 1) * PADN[1]]),
                iota=iota, hm=hm, oma=np.ascontiguousarray(oma),
            )
            for r in (0, 2):
                tb = np.zeros((sched["NT"][r], C), BF)
                u = sched["uniq"][c][r]
                tb[:len(u)] = x_user[u]
                im[f"tb{r}"] = tb
            for r in range(3):
                im[f"wk{r}"] = W[("wk", l, r)]
                im[f"wv{r}"] = W[("wv", l, r)]
                im[f"sidx{r}"] = sched["cores"][c][r]["sidx"]
                im[f"qidx{r}"] = sched["cores"][c][r]["qidx"]
                im[f"dc{r}"] = sched["cores"][c][r]["dc"]
            for t in range(2):
                im[f"wq{t}"] = W[("wq", l, t)]
                im[f"wa{t}"] = W[("wa", l, t)]
            in_maps.append(im)
        res = run_bass_kernel_spmd(nc, in_maps, core_ids, trace=TRACE)
        if TRACE:
            LAST_EXEC_NS.append(res.exec_time_ns)
        global LAST_RES
        LAST_RES = res
        xu_t = np.concatenate([res.results[c]["nx0"] for c in range(M)], axis=0)
        xn_t = np.concatenate([res.results[c]["nx1"] for c in range(M)], axis=0)
        x_user = xu_t[pg[0]]
        x_news = xn_t[pg[1]]

    return np.concatenate([x_user, x_news], axis=0).astype(np.float32)
